# revision 13
# baseline (speedup 1.0000x reference)
"""Trainium2 Bass kernel for nn_CentroidLoss (B=16384, C=2048, D=256).

Strategy (data-parallel over batch across 8 NeuronCores):
  - Each core takes a B/8 = 2048-row shard of preds/labels.
  - Inputs are cast to bf16 on-chip (labels are one-hot 0/1 -> exact in
    bf16; preds lose ~0.4% which is far inside the loss tolerance).
  - Stage A (per core): S_T[d, c] = sum_b preds[b, d] * labels[b, c] via
    PE matmuls (bf16, K=B_local contraction). The >=0.8 mask equals the
    one-hot labels exactly, so no separate mask/count is needed.
  - AllReduce S_T (2 MB) across the 8 cores (on-chip collective).
  - Normalize columns: cn_T = S_T * rsqrt(colsum(S_T^2)) — the class
    count cancels out of the cosine; missing classes give zero columns.
  - Stage B (per core): cos = pn @ cn_T with pn = preds / ||preds||,
    computed as [128, 2048] PSUM tiles; fused reductions give
      s1[b]   = sum_c labels[b,c] * cos[b,c]     (DVE tensor_tensor_reduce)
      H[b]    = sum_c relu(cos[b,c] - 0.7)       (ACT relu + accum)
      R[b]    = relu(s1[b] - 0.7)
  - Host combines per-core partials:
      loss = (1 - sum(s1)/B) + (sum(H) - sum(R)) / max(E-1, 1) / B
    where E = #classes with nonzero column sum (computed on device).
"""

import numpy as np
from contextlib import ExitStack

B, C, D = 16384, 2048, 256
NCORES = 8
BL = B // NCORES          # 2048 rows per core
P = 128
NB = BL // P              # 16 b-tiles per core
ND = D // P               # 2 d-tiles
CH = 512                  # free-dim chunk (one fp32 PSUM bank)
NCH = C // CH             # 4 chunks over C

THR_NEG = -0.7            # bias for relu(cos - 0.7)

_CACHE = {}


def _build_nc():
    from concourse import bacc, tile, mybir, masks

    f32 = mybir.dt.float32
    bf16 = mybir.dt.bfloat16
    AF = mybir.ActivationFunctionType
    OP = mybir.AluOpType

    nc = bacc.Bacc(
        "TRN2", target_bir_lowering=False, debug=False, num_devices=NCORES
    )
    preds_d = nc.dram_tensor("preds", [BL, D], f32, kind="ExternalInput")
    labels_d = nc.dram_tensor("labels", [BL, C], f32, kind="ExternalInput")
    out_d = nc.dram_tensor("partials", [P, 8], f32, kind="ExternalOutput")

    with tile.TileContext(nc) as tc, ExitStack() as ctx:
        const = ctx.enter_context(tc.tile_pool(name="const", bufs=1))
        lab = ctx.enter_context(tc.tile_pool(name="lab", bufs=NB))
        lab32 = ctx.enter_context(tc.tile_pool(name="lab32", bufs=4))
        prd = ctx.enter_context(tc.tile_pool(name="prd", bufs=NB))
        prd32 = ctx.enter_context(tc.tile_pool(name="prd32", bufs=3))
        pnp = ctx.enter_context(tc.tile_pool(name="pnp", bufs=3))
        pnTp = ctx.enter_context(tc.tile_pool(name="pnTp", bufs=ND))
        accp = ctx.enter_context(tc.tile_pool(name="accp", bufs=1))
        stp = ctx.enter_context(tc.tile_pool(name="stp", bufs=ND))
        cnp = ctx.enter_context(tc.tile_pool(name="cnp", bufs=ND))
        rowp = ctx.enter_context(tc.tile_pool(name="rowp", bufs=2))
        scrp = ctx.enter_context(tc.tile_pool(name="scrp", bufs=3))
        sttp = ctx.enter_context(tc.tile_pool(name="sttp", bufs=2))
        dram = ctx.enter_context(tc.tile_pool(name="dram", bufs=1, space="DRAM"))

        # --- constants ---
        ident = const.tile([P, P], bf16)
        masks.make_identity(nc, ident[:])
        ones_col = const.tile([P, 1], bf16)
        nc.vector.memset(ones_col[:], 1.0)
        ones_row = const.tile([1, P], bf16)
        nc.vector.memset(ones_row[:], 1.0)
        bias_neg = const.tile([P, 1], f32)
        nc.vector.memset(bias_neg[:], THR_NEG)
        bias_tiny = const.tile([P, 1], f32)
        nc.vector.memset(bias_tiny[:], 1e-30)

        # --- accumulators ---
        norms = accp.tile([P, 48], f32)   # 0:16 |p|^2, 16:32 |p|, 32:48 1/|p|
        s1a = accp.tile([P, NB], f32)     # per-b-tile s1 columns
        hacc = accp.tile([P, NB], f32)    # per-b-tile H columns
        rlscr = accp.tile([P, NB], f32)   # relu(s1-0.7) scratch
        out_t = accp.tile([P, 8], f32)
        nc.vector.memset(out_t[:], 0.0)

        # --- labels: DMA fp32 -> cast to resident bf16 on GPSIMD ---
        labels_t = []
        for i in range(NB):
            l32 = lab32.tile([P, C], f32, name=f"l32_{i}", tag="l32")
            nc.sync.dma_start(l32[:], labels_d[P * i : P * (i + 1), :])
            lt = lab.tile([P, C], bf16, name=f"lab{i}", tag="lab")
            nc.gpsimd.tensor_copy(lt[:], l32[:])
            labels_t.append(lt)

        # --- preds: DMA + norms + bf16 cast + pn + transpose to pnT ---
        preds_t = []
        pnT = [
            pnTp.tile([P, BL], bf16, name=f"pnT{k}", tag="pnT")
            for k in range(ND)
        ]
        with tc.tile_pool(name="ps_t", bufs=4, space="PSUM") as ps_t:
            for i in range(NB):
                pt = prd32.tile([P, D], f32, name=f"p32_{i}", tag="p32")
                nc.sync.dma_start(pt[:], preds_d[P * i : P * (i + 1), :])
                pb = prd.tile([P, D], bf16, name=f"prb{i}", tag="prb")
                nc.vector.tensor_copy(pb[:], pt[:])
                preds_t.append(pb)
                pnb = pnp.tile([P, D], bf16, name=f"pnb{i}", tag="pnb")
                nc.scalar.activation(
                    pnb[:], pt[:], AF.Square, accum_out=norms[:, i : i + 1]
                )
                nc.scalar.activation(
                    norms[:, 16 + i : 17 + i], norms[:, i : i + 1], AF.Sqrt
                )
                nc.vector.reciprocal(
                    norms[:, 32 + i : 33 + i], norms[:, 16 + i : 17 + i]
                )
                nc.vector.tensor_scalar_mul(
                    pnb[:], pt[:], norms[:, 32 + i : 33 + i]
                )
                for k in range(ND):
                    psT = ps_t.tile(
                        [P, P], bf16, name=f"psT{i}_{k}", tag="psT"
                    )
                    nc.tensor.transpose(
                        psT[:], pnb[:, P * k : P * (k + 1)], ident[:]
                    )
                    nc.scalar.copy(pnT[k][:, P * i : P * (i + 1)], psT[:])

            # --- stage A: S_T = preds^T @ labels (bf16) -> DRAM bounce ---
            st_bounce = dram.tile([D, C], f32, name="st_bounce")
            with (
                tc.tile_pool(name="ps_a", bufs=2, space="PSUM") as ps_a,
                tc.tile_pool(name="st_sb", bufs=2) as st_sb,
            ):
                for m in range(ND):
                    for n in range(NCH):
                        stp_ps = ps_a.tile(
                            [P, CH], f32, name=f"stps{m}_{n}", tag="stps"
                        )
                        for k in range(NB):
                            nc.tensor.matmul(
                                stp_ps[:],
                                preds_t[k][:, P * m : P * (m + 1)],
                                labels_t[k][:, CH * n : CH * (n + 1)],
                                start=(k == 0),
                                stop=(k == NB - 1),
                            )
                        stg = st_sb.tile(
                            [P, CH], f32, name=f"stg{m}_{n}", tag="stg"
                        )
                        nc.vector.tensor_copy(stg[:], stp_ps[:])
                        nc.sync.dma_start(
                            st_bounce[
                                P * m : P * (m + 1), CH * n : CH * (n + 1)
                            ],
                            stg[:],
                        )

        # --- AllReduce S_T across the 8 cores ---
        st_red = dram.tile([D, C], f32, addr_space="Shared", name="st_red")
        nc.gpsimd.collective_compute(
            "AllReduce",
            OP.add,
            replica_groups=[list(range(NCORES))],
            ins=[st_bounce.opt()],
            outs=[st_red.opt()],
        )

        # --- load reduced S_T, normalize columns -> cn_T (bf16) ---
        st_t = []
        for k in range(ND):
            st = stp.tile([P, C], f32, name=f"st{k}", tag="st")
            nc.sync.dma_start(st[:], st_red[P * k : P * (k + 1), :])
            st_t.append(st)

        cn_t = [
            cnp.tile([P, C], bf16, name=f"cn{k}", tag="cn") for k in range(ND)
        ]
        with tc.tile_pool(name="ps_5", bufs=6, space="PSUM") as ps_5:
            sqb = []
            for k in range(ND):
                sb = scrp.tile([P, C], bf16, name=f"sqb{k}", tag="scr")
                nc.scalar.activation(sb[:], st_t[k][:], AF.Square)
                sqb.append(sb)
            ss_row = rowp.tile([1, C], f32, name="ss_row", tag="row")
            for n in range(NCH):
                ssp = ps_5.tile([1, CH], f32, name=f"ssp{n}", tag="ssp", bufs=2)
                for k in range(ND):
                    nc.tensor.matmul(
                        ssp[:],
                        ones_col[:],
                        sqb[k][:, CH * n : CH * (n + 1)],
                        start=(k == 0),
                        stop=(k == ND - 1),
                    )
                nc.vector.tensor_copy(
                    ss_row[0:1, CH * n : CH * (n + 1)], ssp[:]
                )
            # E = #classes with ss > 0  (exactly-zero column <=> missing)
            ex_row = scrp.tile([1, C], bf16, name="ex_row", tag="scr")
            nc.vector.tensor_scalar(
                ex_row[:], ss_row[:], 0.0, None, OP.is_gt, op1=OP.add,
                accum_out=out_t[0:1, 3:4],
            )
            nrm_row = rowp.tile([1, C], f32, name="nrm_row", tag="row")
            nc.scalar.activation(
                nrm_row[:], ss_row[:], AF.Sqrt, bias=bias_tiny[0:1, :]
            )
            r_row = rowp.tile([1, C], f32, name="r_row", tag="row")
            nc.vector.reciprocal(r_row[:], nrm_row[:])
            r_bf = scrp.tile([1, C], bf16, name="r_bf", tag="scr")
            nc.vector.tensor_copy(r_bf[:], r_row[:])
            # broadcast r over partitions via K=1 matmul, then scale S_T
            r_bc = ps_5.tile([P, C], f32, name="r_bc", tag="r_bc", bufs=1)
            for n in range(NCH):
                nc.tensor.matmul(
                    r_bc[:, CH * n : CH * (n + 1)],
                    ones_row[:],
                    r_bf[0:1, CH * n : CH * (n + 1)],
                )
            for k in range(ND):
                nc.vector.tensor_tensor(
                    cn_t[k][:], st_t[k][:], r_bc[:], OP.mult
                )

        # --- stage B: cos tiles + fused reductions ---
        with tc.tile_pool(name="ps_b", bufs=2, space="PSUM") as ps_b:
            for i in range(NB):
                cos = ps_b.tile([P, C], f32, name=f"cos{i}", tag="cos")
                for n in range(NCH):
                    for k in range(ND):
                        nc.tensor.matmul(
                            cos[:, CH * n : CH * (n + 1)],
                            pnT[k][:, P * i : P * (i + 1)],
                            cn_t[k][:, CH * n : CH * (n + 1)],
                            start=(k == 0),
                            stop=(k == ND - 1),
                        )
                hsc = scrp.tile([P, C], bf16, name=f"hsc{i}", tag="scr")
                nc.scalar.activation(
                    hsc[:], cos[:], AF.Relu, bias=bias_neg[:],
                    accum_out=hacc[:, i : i + 1],
                )
                stt = sttp.tile([P, C], f32, name=f"stt{i}", tag="stt")
                nc.vector.scalar_tensor_tensor(
                    out=stt[:],
                    in0=labels_t[i][:],
                    scalar=1.0,
                    in1=cos[:],
                    op0=OP.mult,
                    op1=OP.mult,
                    accum_out=s1a[:, i : i + 1],
                )

        # --- epilogue: per-core partials ---
        nc.scalar.activation(
            rlscr[:], s1a[:], AF.Relu, bias=bias_neg[:],
            accum_out=out_t[:, 2:3],
        )
        nc.vector.tensor_reduce(
            out_t[:, 0:1], s1a[:], mybir.AxisListType.X, OP.add
        )
        nc.vector.tensor_reduce(
            out_t[:, 1:2], hacc[:], mybir.AxisListType.X, OP.add
        )
        nc.sync.dma_start(out_d[:], out_t[:])

    nc.compile()
    return nc


def _get_nc():
    if "nc" not in _CACHE:
        _CACHE["nc"] = _build_nc()
    return _CACHE["nc"]


def _run(in_maps, **kwargs):
    from concourse import bass_utils

    nc = _get_nc()
    return bass_utils.run_bass_kernel_spmd(
        nc, in_maps, core_ids=list(range(NCORES)), **kwargs
    )


def _in_maps(preds, labels):
    preds = np.ascontiguousarray(np.asarray(preds, dtype=np.float32))
    labels = np.ascontiguousarray(np.asarray(labels, dtype=np.float32))
    return [
        {
            "preds": np.ascontiguousarray(preds[c * BL : (c + 1) * BL]),
            "labels": np.ascontiguousarray(labels[c * BL : (c + 1) * BL]),
        }
        for c in range(NCORES)
    ]


def _finalize(results):
    parts = [np.asarray(results[c]["partials"], np.float64) for c in range(NCORES)]
    s1_sum = sum(p[:, 0].sum() for p in parts)
    h_sum = sum(p[:, 1].sum() for p in parts)
    r_sum = sum(p[:, 2].sum() for p in parts)
    e_cnt = parts[0][0, 3]
    loss = (1.0 - s1_sum / B) + (h_sum - r_sum) / max(e_cnt - 1.0, 1.0) / B
    return np.float32(loss)


def kernel(preds, labels):
    res = _run(_in_maps(preds, labels))
    return _finalize(res.results)


if __name__ == "__main__":
    rng = np.random.default_rng(0)
    p = rng.standard_normal((B, D)).astype(np.float32)
    cls = rng.integers(0, C, size=B)
    l = np.zeros((B, C), np.float32)
    l[np.arange(B), cls] = 1.0
    print("loss:", kernel(p, l))


# revision 15
# speedup vs baseline: 1.3359x; 1.3359x over previous
"""Trainium2 Bass kernel for nn_CentroidLoss (B=16384, C=2048, D=256).

Strategy (data-parallel over batch across 8 NeuronCores):
  - Each core takes a B/8 = 2048-row shard of preds/labels, pre-cast to
    bf16 on the host (labels are one-hot 0/1 -> exact in bf16; preds
    lose ~0.4% which is far inside the loss tolerance).
  - Stage A (per core): S_T[d, c] = sum_b preds[b, d] * labels[b, c] via
    PE matmuls (bf16, K=B_local contraction), k-outer so the PE paces
    the labels DMA. The >=0.8 mask equals the one-hot labels exactly,
    so no separate mask/count is needed.
  - AllReduce S_T (2 MB fp32) across the 8 cores (on-chip collective).
  - Normalize columns: cn_T = S_T * rsqrt(colsum(S_T^2)) — the class
    count cancels out of the cosine; missing classes give zero columns.
    rsqrt is computed as exp(-0.5*ln(x)) on ACT (DVE reciprocal is an
    iterative-divide and would serialize on a 1-partition row).
  - Stage B (per core): cos = pn @ cn_T with pn = preds / ||preds||,
    computed as [128, 2048] PSUM tiles; fused reductions give
      s1[b]   = sum_c labels[b,c] * cos[b,c]   (DVE scalar_tensor_tensor)
      H[b]    = sum_c relu(cos[b,c] - 0.7)     (ACT relu + accum)
      R[b]    = relu(s1[b] - 0.7)
  - Host combines per-core partials:
      loss = (1 - sum(s1)/B) + (sum(H) - sum(R)) / max(E-1, 1) / B
    where E = #classes with nonzero column sum (computed on device).
"""

import numpy as np
from contextlib import ExitStack

B, C, D = 16384, 2048, 256
NCORES = 8
BL = B // NCORES          # 2048 rows per core
P = 128
NB = BL // P              # 16 b-tiles per core
ND = D // P               # 2 d-tiles
CH = 512                  # free-dim chunk (one fp32 PSUM bank)
NCH = C // CH             # 4 chunks over C

THR_NEG = -0.7            # bias for relu(cos - 0.7)

_CACHE = {}


def _build_nc():
    from concourse import bacc, tile, mybir, masks

    f32 = mybir.dt.float32
    bf16 = mybir.dt.bfloat16
    AF = mybir.ActivationFunctionType
    OP = mybir.AluOpType

    nc = bacc.Bacc(
        "TRN2", target_bir_lowering=False, debug=False, num_devices=NCORES
    )
    preds_d = nc.dram_tensor("preds", [BL, D], bf16, kind="ExternalInput")
    labels_d = nc.dram_tensor("labels", [BL, C], bf16, kind="ExternalInput")
    out_d = nc.dram_tensor("partials", [P, 8], f32, kind="ExternalOutput")

    with tile.TileContext(nc) as tc, ExitStack() as ctx:
        const = ctx.enter_context(tc.tile_pool(name="const", bufs=1))
        lab = ctx.enter_context(tc.tile_pool(name="lab", bufs=NB))
        prd = ctx.enter_context(tc.tile_pool(name="prd", bufs=NB))
        pnp = ctx.enter_context(tc.tile_pool(name="pnp", bufs=3))
        pnTp = ctx.enter_context(tc.tile_pool(name="pnTp", bufs=ND))
        accp = ctx.enter_context(tc.tile_pool(name="accp", bufs=1))
        stp = ctx.enter_context(tc.tile_pool(name="stp", bufs=ND))
        cnp = ctx.enter_context(tc.tile_pool(name="cnp", bufs=ND))
        rowp = ctx.enter_context(tc.tile_pool(name="rowp", bufs=2))
        scrp = ctx.enter_context(tc.tile_pool(name="scrp", bufs=3))
        sttp = ctx.enter_context(tc.tile_pool(name="sttp", bufs=2))
        dram = ctx.enter_context(tc.tile_pool(name="dram", bufs=1, space="DRAM"))

        # --- constants ---
        ident = const.tile([P, P], bf16)
        masks.make_identity(nc, ident[:])
        ones_col = const.tile([P, 1], bf16)
        nc.vector.memset(ones_col[:], 1.0)
        ones_row = const.tile([1, P], bf16)
        nc.vector.memset(ones_row[:], 1.0)
        bias_neg = const.tile([P, 1], f32)
        nc.vector.memset(bias_neg[:], THR_NEG)
        bias_tiny = const.tile([P, 1], f32)
        nc.vector.memset(bias_tiny[:], 1e-30)

        # --- accumulators ---
        norms = accp.tile([P, 48], f32)   # 0:16 |p|^2, 16:32 ln, 32:48 1/|p|
        s1a = accp.tile([P, NB], f32)     # per-b-tile s1 columns
        hacc = accp.tile([P, NB], f32)    # per-b-tile H columns
        rlscr = accp.tile([P, NB], f32)   # relu(s1-0.7) scratch
        out_t = accp.tile([P, 8], f32)
        nc.vector.memset(out_t[:], 0.0)

        # --- input DMA (both resident, bf16) ---
        labels_t = []
        for i in range(NB):
            lt = lab.tile([P, C], bf16, name=f"lab{i}", tag="lab")
            nc.sync.dma_start(lt[:], labels_d[P * i : P * (i + 1), :])
            labels_t.append(lt)
        preds_t = []
        for i in range(NB):
            pb = prd.tile([P, D], bf16, name=f"prb{i}", tag="prb")
            nc.sync.dma_start(pb[:], preds_d[P * i : P * (i + 1), :])
            preds_t.append(pb)

        # --- norms (vectorized): |p|^2 per tile, then ln, exp(-0.5 ln) ---
        for i in range(NB):
            sqs = pnp.tile([P, D], bf16, name=f"sqs{i}", tag="pnb")
            nc.scalar.activation(
                sqs[:], preds_t[i][:], AF.Square,
                accum_out=norms[:, i : i + 1],
            )
        nc.scalar.activation(norms[:, 16:32], norms[:, 0:16], AF.Ln)
        nc.scalar.activation(
            norms[:, 32:48], norms[:, 16:32], AF.Exp, scale=-0.5
        )

        # --- pn + transpose to pnT (bf16 [D, BL]) ---
        pnT = [
            pnTp.tile([P, BL], bf16, name=f"pnT{k}", tag="pnT")
            for k in range(ND)
        ]
        with tc.tile_pool(name="ps_t", bufs=4, space="PSUM") as ps_t:
            for i in range(NB):
                pnb = pnp.tile([P, D], bf16, name=f"pnb{i}", tag="pnb")
                nc.vector.tensor_scalar_mul(
                    pnb[:], preds_t[i][:], norms[:, 32 + i : 33 + i]
                )
                for k in range(ND):
                    psT = ps_t.tile(
                        [P, P], bf16, name=f"psT{i}_{k}", tag="psT"
                    )
                    nc.tensor.transpose(
                        psT[:], pnb[:, P * k : P * (k + 1)], ident[:]
                    )
                    nc.scalar.copy(pnT[k][:, P * i : P * (i + 1)], psT[:])

        # --- stage A: S_T = preds^T @ labels (bf16, k-outer) -> DRAM ---
        st_bounce = dram.tile([D, C], f32, name="st_bounce")
        with (
            tc.tile_pool(name="ps_a", bufs=8, space="PSUM") as ps_a,
            tc.tile_pool(name="st_sb", bufs=2) as st_sb,
        ):
            st_ps = [
                ps_a.tile(
                    [P, CH], f32, name=f"stps{m}_{n}", tag=f"stps{m}_{n}",
                    bufs=1,
                )
                for m in range(ND)
                for n in range(NCH)
            ]
            for k in range(NB):
                for m in range(ND):
                    for n in range(NCH):
                        nc.tensor.matmul(
                            st_ps[m * NCH + n][:],
                            preds_t[k][:, P * m : P * (m + 1)],
                            labels_t[k][:, CH * n : CH * (n + 1)],
                            start=(k == 0),
                            stop=(k == NB - 1),
                        )
            for m in range(ND):
                for n in range(NCH):
                    stg = st_sb.tile(
                        [P, CH], f32, name=f"stg{m}_{n}", tag="stg"
                    )
                    nc.vector.tensor_copy(stg[:], st_ps[m * NCH + n][:])
                    nc.sync.dma_start(
                        st_bounce[P * m : P * (m + 1), CH * n : CH * (n + 1)],
                        stg[:],
                    )

        # --- AllReduce S_T across the 8 cores ---
        st_red = dram.tile([D, C], f32, addr_space="Shared", name="st_red")
        nc.gpsimd.collective_compute(
            "AllReduce",
            OP.add,
            replica_groups=[list(range(NCORES))],
            ins=[st_bounce.opt()],
            outs=[st_red.opt()],
        )

        # --- load reduced S_T, normalize columns -> cn_T (bf16) ---
        st_t = []
        for k in range(ND):
            st = stp.tile([P, C], f32, name=f"st{k}", tag="st")
            nc.sync.dma_start(st[:], st_red[P * k : P * (k + 1), :])
            st_t.append(st)

        cn_t = [
            cnp.tile([P, C], bf16, name=f"cn{k}", tag="cn") for k in range(ND)
        ]
        with tc.tile_pool(name="ps_5", bufs=6, space="PSUM") as ps_5:
            sqb = []
            for k in range(ND):
                sb = scrp.tile([P, C], bf16, name=f"sqb{k}", tag="scr")
                nc.scalar.activation(sb[:], st_t[k][:], AF.Square)
                sqb.append(sb)
            ss_row = rowp.tile([1, C], f32, name="ss_row", tag="row")
            for n in range(NCH):
                ssp = ps_5.tile([1, CH], f32, name=f"ssp{n}", tag="ssp", bufs=2)
                for k in range(ND):
                    nc.tensor.matmul(
                        ssp[:],
                        ones_col[:],
                        sqb[k][:, CH * n : CH * (n + 1)],
                        start=(k == 0),
                        stop=(k == ND - 1),
                    )
                nc.vector.tensor_copy(
                    ss_row[0:1, CH * n : CH * (n + 1)], ssp[:]
                )
            # E = #classes with ss > 0  (exactly-zero column <=> missing)
            ex_row = scrp.tile([1, C], bf16, name="ex_row", tag="scr")
            nc.vector.tensor_scalar(
                ex_row[:], ss_row[:], 0.0, None, OP.is_gt, op1=OP.add,
                accum_out=out_t[0:1, 3:4],
            )
            # r = rsqrt(ss) = exp(-0.5 * ln(ss + tiny)), bf16 for matmul
            ln_row = rowp.tile([1, C], f32, name="ln_row", tag="row")
            nc.scalar.activation(
                ln_row[:], ss_row[:], AF.Ln, bias=bias_tiny[0:1, :]
            )
            r_bf = scrp.tile([1, C], bf16, name="r_bf", tag="scr")
            nc.scalar.activation(r_bf[:], ln_row[:], AF.Exp, scale=-0.5)
            # broadcast r over partitions via K=1 matmul, then scale S_T
            r_bc = ps_5.tile([P, C], f32, name="r_bc", tag="r_bc", bufs=1)
            for n in range(NCH):
                nc.tensor.matmul(
                    r_bc[:, CH * n : CH * (n + 1)],
                    ones_row[:],
                    r_bf[0:1, CH * n : CH * (n + 1)],
                )
            for k in range(ND):
                nc.vector.tensor_tensor(
                    cn_t[k][:], st_t[k][:], r_bc[:], OP.mult
                )

        # --- stage B: cos tiles + fused reductions ---
        with tc.tile_pool(name="ps_b", bufs=2, space="PSUM") as ps_b:
            for i in range(NB):
                cos = ps_b.tile([P, C], f32, name=f"cos{i}", tag="cos")
                for n in range(NCH):
                    for k in range(ND):
                        nc.tensor.matmul(
                            cos[:, CH * n : CH * (n + 1)],
                            pnT[k][:, P * i : P * (i + 1)],
                            cn_t[k][:, CH * n : CH * (n + 1)],
                            start=(k == 0),
                            stop=(k == ND - 1),
                        )
                hsc = scrp.tile([P, C], bf16, name=f"hsc{i}", tag="scr")
                nc.scalar.activation(
                    hsc[:], cos[:], AF.Relu, bias=bias_neg[:],
                    accum_out=hacc[:, i : i + 1],
                )
                stt = sttp.tile([P, C], f32, name=f"stt{i}", tag="stt")
                nc.vector.scalar_tensor_tensor(
                    out=stt[:],
                    in0=labels_t[i][:],
                    scalar=1.0,
                    in1=cos[:],
                    op0=OP.mult,
                    op1=OP.mult,
                    accum_out=s1a[:, i : i + 1],
                )

        # --- epilogue: per-core partials ---
        nc.scalar.activation(
            rlscr[:], s1a[:], AF.Relu, bias=bias_neg[:],
            accum_out=out_t[:, 2:3],
        )
        nc.vector.tensor_reduce(
            out_t[:, 0:1], s1a[:], mybir.AxisListType.X, OP.add
        )
        nc.vector.tensor_reduce(
            out_t[:, 1:2], hacc[:], mybir.AxisListType.X, OP.add
        )
        nc.sync.dma_start(out_d[:], out_t[:])

    nc.compile()
    return nc


def _get_nc():
    if "nc" not in _CACHE:
        _CACHE["nc"] = _build_nc()
    return _CACHE["nc"]


def _run(in_maps, **kwargs):
    from concourse import bass_utils

    nc = _get_nc()
    return bass_utils.run_bass_kernel_spmd(
        nc, in_maps, core_ids=list(range(NCORES)), **kwargs
    )


def _in_maps(preds, labels):
    import ml_dtypes

    preds = np.asarray(preds, dtype=np.float32).astype(ml_dtypes.bfloat16)
    labels = np.asarray(labels, dtype=np.float32).astype(ml_dtypes.bfloat16)
    return [
        {
            "preds": np.ascontiguousarray(preds[c * BL : (c + 1) * BL]),
            "labels": np.ascontiguousarray(labels[c * BL : (c + 1) * BL]),
        }
        for c in range(NCORES)
    ]


def _finalize(results):
    parts = [np.asarray(results[c]["partials"], np.float64) for c in range(NCORES)]
    s1_sum = sum(p[:, 0].sum() for p in parts)
    h_sum = sum(p[:, 1].sum() for p in parts)
    r_sum = sum(p[:, 2].sum() for p in parts)
    e_cnt = parts[0][0, 3]
    loss = (1.0 - s1_sum / B) + (h_sum - r_sum) / max(e_cnt - 1.0, 1.0) / B
    return np.float32(loss)


def kernel(preds, labels):
    res = _run(_in_maps(preds, labels))
    return _finalize(res.results)


if __name__ == "__main__":
    rng = np.random.default_rng(0)
    p = rng.standard_normal((B, D)).astype(np.float32)
    cls = rng.integers(0, C, size=B)
    l = np.zeros((B, C), np.float32)
    l[np.arange(B), cls] = 1.0
    print("loss:", kernel(p, l))


# revision 20
# speedup vs baseline: 1.4980x; 1.1214x over previous
"""Trainium2 Bass kernel for nn_CentroidLoss (B=16384, C=2048, D=256).

Strategy (data-parallel over batch across 8 NeuronCores):
  - Each core takes a B/8 = 2048-row shard of preds/labels, pre-cast to
    bf16 on the host (labels are one-hot 0/1 -> exact in bf16; preds
    lose ~0.4% which is far inside the loss tolerance).
  - Stage A (per core): S[c, d] = sum_b labels[b, c] * preds[b, d] via
    PE matmuls (bf16, lhsT = labels tile, k-outer so the PE paces the
    labels DMA). The >=0.8 mask equals the one-hot labels exactly.
  - ReduceScatter S (2 MB fp32) -> each core owns a C/8 = 256-class
    shard; normalizes rows with per-partition ops only:
      r = rsqrt(rowsum(S^2)) = exp(-0.5 * ln(ss + tiny))   (ACT)
      cn_sh = S_sh * r  (bf16)
    and counts existing classes in its shard (host sums -> E).
  - AllGather cn (bf16, 1 MB) -> full normalized centroids [C, D];
    transpose on PE to cn_T [D, C] for stage B.
  - Stage B (per core): cos = pn @ cn_T with pn = preds / ||preds||,
    computed as [128, 2048] PSUM tiles; fused reductions give
      s1[b]   = sum_c labels[b,c] * cos[b,c]   (DVE scalar_tensor_tensor)
      H[b]    = sum_c relu(cos[b,c] - 0.7)     (ACT relu + accum)
      R[b]    = relu(s1[b] - 0.7)
  - Host combines per-core partials:
      loss = (1 - sum(s1)/B) + (sum(H) - sum(R)) / max(E-1, 1) / B
"""

import numpy as np
from contextlib import ExitStack

B, C, D = 16384, 2048, 256
NCORES = 8
BL = B // NCORES          # 2048 rows per core
P = 128
NB = BL // P              # 16 b-tiles per core
ND = D // P               # 2 d-tiles
NC = C // P               # 16 c-tiles
CSH = C // NCORES         # 256 classes per core after RS
CH = 512                  # free-dim chunk (one fp32 PSUM bank)
NCH = C // CH             # 4 chunks over C

THR_NEG = -0.7            # bias for relu(cos - 0.7)

_CACHE = {}


def _build_nc():
    from concourse import bacc, tile, mybir, masks

    f32 = mybir.dt.float32
    bf16 = mybir.dt.bfloat16
    AF = mybir.ActivationFunctionType
    OP = mybir.AluOpType

    nc = bacc.Bacc(
        "TRN2", target_bir_lowering=False, debug=False, num_devices=NCORES
    )
    preds_d = nc.dram_tensor("preds", [BL, D], bf16, kind="ExternalInput")
    labels_d = nc.dram_tensor("labels", [BL, C], bf16, kind="ExternalInput")
    out_d = nc.dram_tensor("partials", [P, 8], f32, kind="ExternalOutput")

    with tile.TileContext(nc) as tc, ExitStack() as ctx:
        const = ctx.enter_context(tc.tile_pool(name="const", bufs=1))
        lab = ctx.enter_context(tc.tile_pool(name="lab", bufs=NB))
        prd = ctx.enter_context(tc.tile_pool(name="prd", bufs=NB))
        pnp = ctx.enter_context(tc.tile_pool(name="pnp", bufs=3))
        pnTp = ctx.enter_context(tc.tile_pool(name="pnTp", bufs=ND))
        accp = ctx.enter_context(tc.tile_pool(name="accp", bufs=1))
        cnp = ctx.enter_context(tc.tile_pool(name="cnp", bufs=ND))
        scrp = ctx.enter_context(tc.tile_pool(name="scrp", bufs=3))
        sttp = ctx.enter_context(tc.tile_pool(name="sttp", bufs=2))
        dram = ctx.enter_context(tc.tile_pool(name="dram", bufs=1, space="DRAM"))

        # --- constants ---
        ident = const.tile([P, P], bf16)
        masks.make_identity(nc, ident[:])
        bias_neg = const.tile([P, 1], f32)
        nc.vector.memset(bias_neg[:], THR_NEG)
        bias_tiny = const.tile([P, 1], f32)
        nc.vector.memset(bias_tiny[:], 1e-30)

        # --- accumulators ---
        norms = accp.tile([P, 48], f32)   # 0:16 |p|^2, 16:32 ln, 32:48 rsqrt
        s1a = accp.tile([P, NB], f32)     # per-b-tile s1 columns
        hacc = accp.tile([P, NB], f32)    # per-b-tile H columns
        rlscr = accp.tile([P, NB], f32)   # relu(s1-0.7) scratch
        out_t = accp.tile([P, 8], f32)
        nc.vector.memset(out_t[:], 0.0)

        # --- input DMA: preds first (small, unblocks norms/transposes) ---
        preds_t = []
        for i in range(NB):
            pb = prd.tile([P, D], bf16, name=f"prb{i}", tag="prb")
            nc.sync.dma_start(pb[:], preds_d[P * i : P * (i + 1), :])
            preds_t.append(pb)
        labels_t = []
        for i in range(NB):
            lt = lab.tile([P, C], bf16, name=f"lab{i}", tag="lab")
            nc.sync.dma_start(lt[:], labels_d[P * i : P * (i + 1), :])
            labels_t.append(lt)

        # --- norms: |p|^2 per tile, then rsqrt = exp(-0.5 ln) ---
        for i in range(NB):
            sqs = pnp.tile([P, D], bf16, name=f"sqs{i}", tag="pnb")
            nc.scalar.activation(
                sqs[:], preds_t[i][:], AF.Square,
                accum_out=norms[:, i : i + 1],
            )
        nc.scalar.activation(norms[:, 16:32], norms[:, 0:16], AF.Ln)
        nc.scalar.activation(
            norms[:, 32:48], norms[:, 16:32], AF.Exp, scale=-0.5
        )

        # --- pn + transpose to pnT (bf16 [D, BL]) ---
        pnT = [
            pnTp.tile([P, BL], bf16, name=f"pnT{k}", tag="pnT")
            for k in range(ND)
        ]
        with tc.tile_pool(name="ps_t", bufs=4, space="PSUM") as ps_t:
            for i in range(NB):
                pnb = pnp.tile([P, D], bf16, name=f"pnb{i}", tag="pnb")
                nc.vector.tensor_scalar_mul(
                    pnb[:], preds_t[i][:], norms[:, 32 + i : 33 + i]
                )
                for k in range(ND):
                    psT = ps_t.tile(
                        [P, P], bf16, name=f"psT{i}_{k}", tag="psT"
                    )
                    nc.tensor.transpose(
                        psT[:], pnb[:, P * k : P * (k + 1)], ident[:]
                    )
                    nc.scalar.copy(pnT[k][:, P * i : P * (i + 1)], psT[:])

        # --- stage A: S[c, d] = labels^T @ preds (bf16, k-outer) -> DRAM
        # 16 c-tiles packed 2-per-PSUM-bank ([128, 512] = two [128, 256]).
        s_bounce = dram.tile([C, D], f32, name="s_bounce")
        with (
            tc.tile_pool(name="ps_a", bufs=8, space="PSUM") as ps_a,
            tc.tile_pool(name="st_sb", bufs=4) as st_sb,
        ):
            for sweep in range(2):
                s_ps = [
                    ps_a.tile(
                        [P, D], f32, name=f"sps{sweep}_{j}", tag=f"sps{j}",
                        bufs=1,
                    )
                    for j in range(8)
                ]
                for k in range(NB):
                    for j in range(8):
                        t = sweep * 8 + j
                        nc.tensor.matmul(
                            s_ps[j][:],
                            labels_t[k][:, P * t : P * (t + 1)],
                            preds_t[k][:],
                            start=(k == 0),
                            stop=(k == NB - 1),
                        )
                for j in range(8):
                    t = sweep * 8 + j
                    stg = st_sb.tile([P, D], f32, name=f"stg{t}", tag="stg")
                    nc.vector.tensor_copy(stg[:], s_ps[j][:])
                    nc.sync.dma_start(
                        s_bounce[P * t : P * (t + 1), :], stg[:]
                    )

        # --- ReduceScatter S: each core owns classes [256*rank, +256) ---
        rs_out = dram.tile([CSH, D], f32, name="rs_out")
        nc.gpsimd.collective_compute(
            "ReduceScatter",
            OP.add,
            replica_groups=[list(range(NCORES))],
            ins=[s_bounce.opt()],
            outs=[rs_out.opt()],
        )

        # --- normalize the local shard: cn_sh = S_sh * rsqrt(ss) ---
        ag_in = dram.tile([CSH, D], bf16, name="ag_in")
        with tc.tile_pool(name="shp", bufs=4) as shp:
            for j in range(CSH // P):
                ssh = shp.tile([P, D], f32, name=f"ssh{j}", tag="ssh")
                nc.sync.dma_start(ssh[:], rs_out[P * j : P * (j + 1), :])
                sq = shp.tile([P, D], bf16, name=f"shsq{j}", tag="shsq")
                nc.scalar.activation(
                    sq[:], ssh[:], AF.Square,
                    accum_out=norms[:, 16 + j : 17 + j],
                )
                # count existing classes in shard (ss > 0), host sums -> E
                exs = shp.tile([P, 1], bf16, name=f"exs{j}", tag="exs")
                nc.vector.tensor_scalar(
                    exs[:], norms[:, 16 + j : 17 + j], 0.0, None,
                    OP.is_gt, op1=OP.add, accum_out=out_t[:, 4 + j : 5 + j],
                )
                nc.scalar.activation(
                    norms[:, 18 + j : 19 + j], norms[:, 16 + j : 17 + j],
                    AF.Ln, bias=bias_tiny[:],
                )
                nc.scalar.activation(
                    norms[:, 20 + j : 21 + j], norms[:, 18 + j : 19 + j],
                    AF.Exp, scale=-0.5,
                )
                cnsh = shp.tile([P, D], bf16, name=f"cnsh{j}", tag="cnsh")
                nc.vector.tensor_scalar_mul(
                    cnsh[:], ssh[:], norms[:, 20 + j : 21 + j]
                )
                nc.sync.dma_start(ag_in[P * j : P * (j + 1), :], cnsh[:])

        # --- AllGather cn (bf16) -> [C, D], then transpose to cn_T ---
        ag_out = dram.tile([C, D], bf16, addr_space="Shared", name="ag_out")
        nc.gpsimd.collective_compute(
            "AllGather",
            OP.bypass,
            replica_groups=[list(range(NCORES))],
            ins=[ag_in.opt()],
            outs=[ag_out.opt()],
        )

        cn_t = [
            cnp.tile([P, C], bf16, name=f"cn{k}", tag="cn") for k in range(ND)
        ]
        with (
            tc.tile_pool(name="ps_c", bufs=4, space="PSUM") as ps_c,
            tc.tile_pool(name="cnl", bufs=4) as cnl,
        ):
            for t in range(NC):
                cl = cnl.tile([P, D], bf16, name=f"cnl{t}", tag="cnl")
                nc.sync.dma_start(cl[:], ag_out[P * t : P * (t + 1), :])
                for k in range(ND):
                    psC = ps_c.tile([P, P], bf16, name=f"psC{t}_{k}", tag="psC")
                    nc.tensor.transpose(
                        psC[:], cl[:, P * k : P * (k + 1)], ident[:]
                    )
                    nc.vector.tensor_copy(
                        cn_t[k][:, P * t : P * (t + 1)], psC[:]
                    )

        # --- stage B: cos tiles + fused reductions ---
        with tc.tile_pool(name="ps_b", bufs=2, space="PSUM") as ps_b:
            for i in range(NB):
                cos = ps_b.tile([P, C], f32, name=f"cos{i}", tag="cos")
                for n in range(NCH):
                    for k in range(ND):
                        nc.tensor.matmul(
                            cos[:, CH * n : CH * (n + 1)],
                            pnT[k][:, P * i : P * (i + 1)],
                            cn_t[k][:, CH * n : CH * (n + 1)],
                            start=(k == 0),
                            stop=(k == ND - 1),
                        )
                hsc = scrp.tile([P, C], bf16, name=f"hsc{i}", tag="scr")
                nc.scalar.activation(
                    hsc[:], cos[:], AF.Relu, bias=bias_neg[:],
                    accum_out=hacc[:, i : i + 1],
                )
                stt = sttp.tile([P, C], f32, name=f"stt{i}", tag="stt")
                nc.vector.scalar_tensor_tensor(
                    out=stt[:],
                    in0=labels_t[i][:],
                    scalar=1.0,
                    in1=cos[:],
                    op0=OP.mult,
                    op1=OP.mult,
                    accum_out=s1a[:, i : i + 1],
                )

        # --- epilogue: per-core partials ---
        nc.scalar.activation(
            rlscr[:], s1a[:], AF.Relu, bias=bias_neg[:],
            accum_out=out_t[:, 2:3],
        )
        nc.vector.tensor_reduce(
            out_t[:, 0:1], s1a[:], mybir.AxisListType.X, OP.add
        )
        nc.vector.tensor_reduce(
            out_t[:, 1:2], hacc[:], mybir.AxisListType.X, OP.add
        )
        nc.sync.dma_start(out_d[:], out_t[:])

    nc.compile()
    return nc


def _get_nc():
    if "nc" not in _CACHE:
        _CACHE["nc"] = _build_nc()
    return _CACHE["nc"]


def _run(in_maps, **kwargs):
    from concourse import bass_utils

    nc = _get_nc()
    return bass_utils.run_bass_kernel_spmd(
        nc, in_maps, core_ids=list(range(NCORES)), **kwargs
    )


def _in_maps(preds, labels):
    import ml_dtypes

    preds = np.asarray(preds, dtype=np.float32).astype(ml_dtypes.bfloat16)
    labels = np.asarray(labels, dtype=np.float32).astype(ml_dtypes.bfloat16)
    return [
        {
            "preds": np.ascontiguousarray(preds[c * BL : (c + 1) * BL]),
            "labels": np.ascontiguousarray(labels[c * BL : (c + 1) * BL]),
        }
        for c in range(NCORES)
    ]


def _finalize(results):
    parts = [np.asarray(results[c]["partials"], np.float64) for c in range(NCORES)]
    s1_sum = sum(p[:, 0].sum() for p in parts)
    h_sum = sum(p[:, 1].sum() for p in parts)
    r_sum = sum(p[:, 2].sum() for p in parts)
    e_cnt = sum(p[:, 4].sum() + p[:, 5].sum() for p in parts)
    loss = (1.0 - s1_sum / B) + (h_sum - r_sum) / max(e_cnt - 1.0, 1.0) / B
    return np.float32(loss)


def kernel(preds, labels):
    res = _run(_in_maps(preds, labels))
    return _finalize(res.results)


if __name__ == "__main__":
    rng = np.random.default_rng(0)
    p = rng.standard_normal((B, D)).astype(np.float32)
    cls = rng.integers(0, C, size=B)
    l = np.zeros((B, C), np.float32)
    l[np.arange(B), cls] = 1.0
    print("loss:", kernel(p, l))
